# revision 22
# baseline (speedup 1.0000x reference)
"""Trainium2 Bass kernel for a 4-layer dense transformer (CustomGPT1).

Full-input contract: kernel(**inputs) takes the unsharded inputs (B=16),
shards batch across 8 NeuronCores (2 examples/core, data-parallel; params
replicated), runs one SPMD Bass kernel, and gathers the full output.

Layout strategy per core/example:
- LN output h kept both s-major (h_nat, rhs of probs@h) and d-major (hT, both
  operands of h@h^T) in fp32r; scores computed directly in [t, s] orientation
  so the exp tiles feed probs@h with no transposes of probs.
- Softmax without max-subtraction (|scores| <= sqrt(D) by Cauchy-Schwarz after
  LN, exp fits fp32 comfortably); denominators accumulated on DVE and reduced
  across partitions with a single ones-matmul per s-chunk.
- FFN computed in f-major orientation; all matmul operands fp32r.
"""
import sys
sys.path.insert(0, "/opt/trn_rl_repo")
import math
import numpy as np
import concourse.bass as bass
import concourse.mybir as mybir
import concourse.tile as tile
from concourse import bacc
from concourse.bass_utils import run_bass_kernel_spmd
from concourse.masks import make_identity

F32 = mybir.dt.float32
F32R = mybir.dt.float32r
I32 = mybir.dt.int32
AF = mybir.ActivationFunctionType
OP = mybir.AluOpType

B, S, D, L, FF, V = 16, 2048, 512, 4, 2048, 33
NCORES, BL = 8, B // 8          # 2 examples per core
P = 128
SB = S // P                     # 16 s-blocks per example
DC = D // P                     # 4 d-chunks
FC = FF // P                    # 16 f-chunks
ASC = 512                       # attention s-chunk width
NASC = S // ASC                 # 4
ASB = ASC // P                  # 4 s-blocks per attention chunk
FSC = 512                       # ffn s-chunk width
NFSC = S // FSC                 # 4
SCALE = 1.0 / math.sqrt(D)
EPS = 1e-5
VP = V + 1                      # psum-friendly padded vocab


def build(simple):
    nc = bacc.Bacc(None, target_bir_lowering=False)

    ids = nc.dram_tensor("ids", [BL, S], I32, kind="ExternalInput")
    aidx = nc.dram_tensor("aidx", [BL], I32, kind="ExternalInput")
    mask = nc.dram_tensor("mask", [BL, S], F32, kind="ExternalInput")
    tok_emb = nc.dram_tensor("tok_emb", [V, D], F32, kind="ExternalInput")
    pos_emb = nc.dram_tensor("pos_emb", [S, D], F32, kind="ExternalInput")
    attr_emb = nc.dram_tensor("attr_emb", [608, D], F32, kind="ExternalInput")
    lnw = nc.dram_tensor("lnw", [L, D], F32, kind="ExternalInput")
    lnb = nc.dram_tensor("lnb", [L, D], F32, kind="ExternalInput")
    w1 = nc.dram_tensor("w1", [L, D, FF], F32, kind="ExternalInput")
    b1 = nc.dram_tensor("b1", [L, FF], F32, kind="ExternalInput")
    w2 = nc.dram_tensor("w2", [L, FF, D], F32, kind="ExternalInput")
    b2 = nc.dram_tensor("b2", [L, D], F32, kind="ExternalInput")
    out_w = nc.dram_tensor("out_w", [D, V], F32, kind="ExternalInput")
    out_b = nc.dram_tensor("out_b", [V], F32, kind="ExternalInput")
    out = nc.dram_tensor("out", [BL, S, V], F32, kind="ExternalOutput")

    def bcast_row(handle, offset, n):
        # [n]-vector at element `offset`, replicated across all 128 partitions
        return bass.AP(tensor=handle.ap().tensor, offset=offset, ap=[[0, P], [1, n]])

    with tile.TileContext(nc) as tc:
        with tc.tile_pool(name="cst", bufs=1) as cst, \
             tc.tile_pool(name="parw", bufs=2) as parw, \
             tc.tile_pool(name="wts", bufs=1) as wts, \
             tc.tile_pool(name="big", bufs=1) as big, \
             tc.tile_pool(name="tr4", bufs=1) as tr4, \
             tc.tile_pool(name="tmp", bufs=2) as tmp, \
             tc.tile_pool(name="sml", bufs=6) as sml, \
             tc.tile_pool(name="dram", bufs=1, space="DRAM") as dram, \
             tc.tile_pool(name="pb", bufs=6, space="PSUM") as pb, \
             tc.tile_pool(name="tpr", bufs=2, space="PSUM") as tpr:

            xbuf = dram.tile([BL, S, D], F32, tag="xbuf")
            abuf = dram.tile([BL, S, D], F32, tag="abuf")

            # constants
            ident_f = cst.tile([P, P], F32, tag="identf")
            make_identity(nc, ident_f)
            ident_r = cst.tile([P, P], F32R, tag="identr")
            nc.vector.tensor_copy(ident_r, ident_f)
            eps_t = cst.tile([P, 1], F32, tag="eps")
            nc.vector.memset(eps_t, EPS)
            ones_f = cst.tile([P, 2], F32, tag="onesf")
            nc.vector.memset(ones_f, 1.0)
            ones_r = cst.tile([P, 2], F32R, tag="onesr")
            nc.vector.tensor_copy(ones_r, ones_f)
            outb_b = cst.tile([P, V], F32, tag="outb")
            nc.sync.dma_start(out=outb_b, in_=bcast_row(out_b, 0, V))
            outw_st = cst.tile([P, DC, VP], F32, tag="outwst")
            nc.vector.memset(outw_st, 0.0)
            nc.sync.dma_start(out=outw_st[:, :, :V], in_=out_w.ap().rearrange("(do p) v -> p do v", p=P))
            outw_sb = cst.tile([P, DC, VP], F32R, tag="outw")
            nc.vector.tensor_copy(outw_sb, outw_st)

            # per-example mask bias: (m - 1) * 1e9  ==  (1 - m) * (-1e9), layout [t_in=128, tc=16]
            maskb = []
            for b in range(BL):
                ml = sml.tile([P, SB], F32, tag=f"mload{b}")
                nc.sync.dma_start(out=ml, in_=mask.ap()[b].rearrange("(tc p) -> p tc", p=P))
                mb = cst.tile([P, SB], F32, tag=f"maskb{b}")
                nc.vector.tensor_scalar(out=mb, in0=ml, scalar1=1.0, scalar2=1e9,
                                        op0=OP.subtract, op1=OP.mult)
                maskb.append(mb)

            # per-example attribute-embedding rows (replicated over partitions)
            attrvs = []
            for b in range(BL):
                ai = sml.tile([P, 1], I32, tag="aidx", name=f"ai{b}")
                nc.sync.dma_start(out=ai, in_=bass.AP(tensor=aidx.ap().tensor, offset=b, ap=[[0, P], [1, 1]]))
                attrv = cst.tile([P, D], F32, tag=f"attrv{b}")
                nc.gpsimd.indirect_dma_start(
                    out=attrv[:, :], out_offset=None, in_=attr_emb[:, :],
                    in_offset=bass.IndirectOffsetOnAxis(ap=ai[:, :1], axis=0))
                attrvs.append(attrv)

            def embed_tile(b, sb):
                """Compute x = tok_emb[ids] + pos_emb + attr row for one s-block."""
                r0 = sb * P
                it = sml.tile([P, 1], I32, tag="ids", name="it")
                nc.sync.dma_start(out=it, in_=ids.ap()[b, r0:r0 + P].rearrange("(p one) -> p one", one=1))
                tokv = tmp.tile([P, D], F32, tag="tokv", name="tokv")
                nc.gpsimd.indirect_dma_start(
                    out=tokv[:, :], out_offset=None, in_=tok_emb[:, :],
                    in_offset=bass.IndirectOffsetOnAxis(ap=it[:, :1], axis=0))
                xe = tmp.tile([P, D], F32, tag="xld", bufs=3, name="xe")
                nc.sync.dma_start(out=xe, in_=pos_emb.ap()[r0:r0 + P, :])
                nc.vector.tensor_tensor(out=xe, in0=xe, in1=tokv, op=OP.add)
                nc.vector.tensor_tensor(out=xe, in0=xe, in1=attrvs[b], op=OP.add)
                nc.sync.dma_start(out=xbuf[b, r0:r0 + P, :], in_=xe)
                return xe

            def ln_stats(src_dram, b, embed=False):
                """Pass 1 of LayerNorm over a [S, D] example: per-block bn stats,
                one batched sqrt+reciprocal (single ACT table load per phase)."""
                mvall = sml.tile([P, SB, 2], F32, tag="mvall", bufs=2, name="mvall")
                for sb in range(SB):
                    r0 = sb * P
                    if embed:
                        xt = embed_tile(b, sb)
                    else:
                        xt = tmp.tile([P, D], F32, tag="xld", bufs=3, name="xs")
                        nc.sync.dma_start(out=xt, in_=src_dram[b, r0:r0 + P, :])
                    st = sml.tile([P, 6], F32, tag="st", name="st")
                    nc.vector.bn_stats(st, xt)
                    nc.vector.bn_aggr(mvall[:, sb, :], st)
                rstdall = sml.tile([P, SB], F32, tag="rstdall", bufs=2, name="rstdall")
                nc.scalar.activation(rstdall, mvall[:, :, 1:2], AF.Sqrt, bias=eps_t, scale=1.0)
                nc.vector.reciprocal(rstdall, rstdall)
                return mvall, rstdall

            def ln_apply(xt, mvall, rstdall, sb, lnw_b, lnb_b, out_slice):
                nc.vector.tensor_scalar(out=out_slice, in0=xt, scalar1=mvall[:, sb, 0:1],
                                        scalar2=rstdall[:, sb:sb + 1],
                                        op0=OP.subtract, op1=OP.mult)
                if not simple:
                    nc.vector.tensor_tensor(out=out_slice, in0=out_slice, in1=lnw_b, op=OP.mult)
                    nc.vector.tensor_tensor(out=out_slice, in0=out_slice, in1=lnb_b, op=OP.add)

            def transpose_to(dst, src_tile, sb):
                """PE-transpose [s128, D] fp32r tile into dst[:, :, sb*128:(sb+1)*128]."""
                r0 = sb * P
                pt = tpr.tile([P, D], F32R, tag="tpr", name="pt")
                for dc in range(DC):
                    nc.tensor.transpose(pt[:, dc * P:(dc + 1) * P], src_tile[:, dc * P:(dc + 1) * P], ident_r)
                nc.vector.tensor_copy(dst[:, :, r0:r0 + P], pt.rearrange("p (dc q) -> p dc q", q=P))

            # ---------------- layers ----------------
            for l in range(L):
                w1sb = wts.tile([P, DC, FF], F32R, tag="w1")
                nc.gpsimd.dma_start(out=w1sb, in_=w1.ap()[l].rearrange("(do p) f -> p do f", p=P))
                w2sb = wts.tile([P, FC, D], F32R, tag="w2")
                nc.gpsimd.dma_start(out=w2sb, in_=w2.ap()[l].rearrange("(fc p) d -> p fc d", p=P))
                if not simple:
                    lnw_b = parw.tile([P, D], F32, tag="lnw")
                    nc.sync.dma_start(out=lnw_b, in_=bcast_row(lnw, l * D, D))
                    lnb_b = parw.tile([P, D], F32, tag="lnb")
                    nc.sync.dma_start(out=lnb_b, in_=bcast_row(lnb, l * D, D))
                    b2sb = parw.tile([P, DC], F32, tag="b2")
                    nc.sync.dma_start(out=b2sb, in_=b2.ap()[l].rearrange("(dc p) -> p dc", p=P))
                else:
                    lnw_b = lnb_b = b2sb = None
                b1sb = parw.tile([P, FC], F32, tag="b1")
                nc.sync.dma_start(out=b1sb, in_=b1.ap()[l].rearrange("(fc p) -> p fc", p=P))

                for b in range(BL):
                    # ---- LN1 (+ embeddings on layer 0): -> h_nat (s-major) + hT (d-major)
                    h_nat = big.tile([P, SB, D], F32R, tag="h")
                    hT = tr4.tile([P, DC, S], F32R, tag="tr4", name="hT")
                    mvall, rstdall = ln_stats(xbuf, b, embed=(l == 0))
                    for sb in range(SB):
                        r0 = sb * P
                        xt = tmp.tile([P, D], F32, tag="xld", bufs=3, name="xt")
                        nc.sync.dma_start(out=xt, in_=xbuf[b, r0:r0 + P, :])
                        hs = h_nat[:, sb, :]
                        ln_apply(xt, mvall, rstdall, sb, lnw_b, lnb_b, hs)
                        transpose_to(hT, hs, sb)

                    # ---- attention: scoresT -> exp -> attnU accumulation; DVE denominators
                    for sc in range(NASC):
                        c0 = sc * ASC
                        pa = [pb.tile([P, FSC], F32, tag="pb", name=f"pa{_h}") for _h in range(ASB)]
                        dacc = tmp.tile([P, ASC], F32R, tag="dacc", name="dacc")
                        for tc_i in range(SB):
                            ps_sc = pb.tile([P, ASC], F32, tag="pb", name="ps_sc")
                            for do in range(DC):
                                nc.tensor.matmul(ps_sc, hT[:, do, tc_i * P:(tc_i + 1) * P],
                                                 hT[:, do, c0:c0 + ASC],
                                                 start=(do == 0), stop=(do == DC - 1))
                            et = tmp.tile([P, ASC], F32R, tag="expt", bufs=3, name="et")
                            nc.scalar.activation(et, ps_sc, AF.Exp,
                                                 bias=maskb[b][:, tc_i:tc_i + 1], scale=SCALE)
                            if tc_i == 0:
                                nc.vector.tensor_copy(dacc, et)
                            else:
                                nc.vector.tensor_tensor(out=dacc, in0=dacc, in1=et, op=OP.add)
                            for hf in range(ASB):
                                nc.tensor.matmul(pa[hf][:, :], et[:, hf * P:(hf + 1) * P],
                                                 h_nat[:, tc_i, :],
                                                 start=(tc_i == 0), stop=(tc_i == SB - 1))
                        # denominators: reduce dacc over partitions with a ones-matmul,
                        # then reshape the [1, ASC] row into per-partition scalars [128, ASB]
                        pden = pb.tile([1, ASC], F32, tag="pb", name="pden")
                        nc.tensor.matmul(pden, ones_r[:, :1], dacc, start=True, stop=True)
                        drow = sml.tile([1, ASC], F32, tag="drow", bufs=2, name="drow")
                        nc.vector.tensor_copy(drow, pden)
                        dsb = sml.tile([P, ASB], F32, tag="dsb", name="dsb")
                        for _hf in range(ASB):
                            nc.sync.dma_start(
                                out=dsb[:, _hf:_hf + 1],
                                in_=drow[0:1, _hf * P:(_hf + 1) * P].rearrange("one (p o) -> one p o", o=1))
                        drec = sml.tile([P, ASB], F32, tag="drec", name="drec")
                        nc.vector.reciprocal(drec, dsb)
                        for hf in range(ASB):
                            r0 = c0 + hf * P
                            at = tmp.tile([P, D], F32, tag="attn", name="at")
                            nc.vector.tensor_scalar(out=at, in0=pa[hf][:, :], scalar1=drec[:, hf:hf + 1],
                                                    scalar2=None, op0=OP.mult)
                            xr = tmp.tile([P, D], F32, tag="xres", name="xr")
                            nc.sync.dma_start(out=xr, in_=xbuf[b, r0:r0 + P, :])
                            nc.vector.tensor_tensor(out=at, in0=at, in1=xr, op=OP.add)
                            nc.sync.dma_start(out=abuf[b, r0:r0 + P, :], in_=at)

                    # ---- LN2: abuf -> n2T (d-major, fp32r)
                    n2T = tr4.tile([P, DC, S], F32R, tag="tr4", name="n2T")
                    mvall2, rstdall2 = ln_stats(abuf, b)
                    for sb in range(SB):
                        r0 = sb * P
                        xt = tmp.tile([P, D], F32, tag="xld", bufs=3, name="xt2")
                        nc.sync.dma_start(out=xt, in_=abuf[b, r0:r0 + P, :])
                        n2 = tmp.tile([P, D], F32R, tag="n2", name="n2")
                        ln_apply(xt, mvall2, rstdall2, sb, lnw_b, lnb_b, n2[:, :])
                        transpose_to(n2T, n2, sb)

                    # ---- FFN (f-major): ff = gelu(n2 @ w1 + b1); x = ff @ w2 + b2 + attn
                    last = (l == L - 1)
                    for fs in range(NFSC):
                        c0 = fs * FSC
                        p2 = [pb.tile([P, FSC], F32, tag="pb", name=f"p2_{_d}") for _d in range(DC)]
                        for fc in range(FC):
                            pf = pb.tile([P, FSC], F32, tag="pb", name="pf")
                            for do in range(DC):
                                nc.tensor.matmul(pf, w1sb[:, do, fc * P:(fc + 1) * P],
                                                 n2T[:, do, c0:c0 + FSC],
                                                 start=(do == 0), stop=(do == DC - 1))
                            fg = tmp.tile([P, FSC], F32R, tag="ffg", bufs=3, name="fg")
                            nc.scalar.activation(fg, pf, AF.Gelu, bias=b1sb[:, fc:fc + 1], scale=1.0)
                            for dc in range(DC):
                                nc.tensor.matmul(p2[dc], w2sb[:, fc, dc * P:(dc + 1) * P], fg,
                                                 start=(fc == 0), stop=(fc == FC - 1))
                        f2sb = []
                        for dc in range(DC):
                            t = tmp.tile([P, FSC], F32R, tag="f2sb", bufs=4, name=f"f2sb{dc}")
                            if simple:
                                nc.vector.tensor_copy(t, p2[dc])
                            else:
                                nc.vector.tensor_scalar(out=t, in0=p2[dc], scalar1=b2sb[:, dc:dc + 1],
                                                        scalar2=None, op0=OP.add)
                            f2sb.append(t)
                        for sbi in range(FSC // P):
                            r0 = c0 + sbi * P
                            px = pb.tile([P, D], F32R, tag="pb", name="px")
                            for dc in range(DC):
                                nc.tensor.transpose(px[:, dc * P:(dc + 1) * P],
                                                    f2sb[dc][:, sbi * P:(sbi + 1) * P], ident_r)
                            ar = tmp.tile([P, D], F32, tag="ares", name="ar")
                            nc.sync.dma_start(out=ar, in_=abuf[b, r0:r0 + P, :])
                            if not last:
                                xn = tmp.tile([P, D], F32, tag="xn", name="xn")
                                nc.vector.tensor_tensor(out=xn, in0=px, in1=ar, op=OP.add)
                                nc.sync.dma_start(out=xbuf[b, r0:r0 + P, :], in_=xn)
                            else:
                                # final projection fused into the epilogue
                                xnr = tmp.tile([P, D], F32R, tag="xn", name="xnr")
                                nc.vector.tensor_tensor(out=xnr, in0=px, in1=ar, op=OP.add)
                                pt = tpr.tile([P, D], F32R, tag="tpr", name="ptx")
                                for dc in range(DC):
                                    nc.tensor.transpose(pt[:, dc * P:(dc + 1) * P],
                                                        xnr[:, dc * P:(dc + 1) * P], ident_r)
                                xtsb = tmp.tile([P, DC, P], F32R, tag="xtsb", name="xtsb")
                                nc.vector.tensor_copy(xtsb, pt.rearrange("p (dc q) -> p dc q", q=P))
                                po = pb.tile([P, VP], F32, tag="pb", name="po")
                                for do in range(DC):
                                    nc.tensor.matmul(po, xtsb[:, do, :], outw_sb[:, do, :],
                                                     start=(do == 0), stop=(do == DC - 1))
                                ot = tmp.tile([P, V], F32, tag="ot", name="ot")
                                nc.vector.tensor_tensor(out=ot, in0=po[:, :V], in1=outb_b, op=OP.add)
                                nc.sync.dma_start(out=out[b, r0:r0 + P, :], in_=ot)

    nc.compile()
    return nc


_NC = {}


def _get_nc(simple=True):
    if simple not in _NC:
        _NC[simple] = build(simple)
    return _NC[simple]


def _is_simple(inputs):
    return (np.all(np.asarray(inputs["ln_w"]) == 1.0)
            and np.all(np.asarray(inputs["ln_b"]) == 0.0)
            and np.all(np.asarray(inputs["b2"]) == 0.0))


def make_in_maps(inputs):
    f = lambda a: np.ascontiguousarray(np.asarray(a, dtype=np.float32))
    i = lambda a: np.ascontiguousarray(np.asarray(a, dtype=np.int32))
    shared = {
        "tok_emb": f(inputs["tok_emb"]), "pos_emb": f(inputs["pos_emb"]),
        "attr_emb": f(inputs["attr_emb"]),
        "lnw": f(inputs["ln_w"]), "lnb": f(inputs["ln_b"]),
        "w1": f(inputs["w1"]), "b1": f(inputs["b1"]),
        "w2": f(inputs["w2"]), "b2": f(inputs["b2"]),
        "out_w": f(inputs["out_w"]), "out_b": f(inputs["out_b"]),
    }
    in_maps = []
    for c in range(NCORES):
        sl = slice(BL * c, BL * (c + 1))
        m = dict(shared)
        m["ids"] = i(inputs["input_ids"][sl])
        m["aidx"] = i(inputs["combined_indices"][sl])
        m["mask"] = f(inputs["attention_mask"][sl])
        in_maps.append(m)
    return in_maps


def kernel(**inputs):
    res = run_bass_kernel_spmd(_get_nc(_is_simple(inputs)), make_in_maps(inputs),
                               core_ids=list(range(NCORES)))
    return np.concatenate([r["out"] for r in res.results], axis=0)


# revision 31
# speedup vs baseline: 1.0123x; 1.0123x over previous
"""Trainium2 Bass kernel for a 4-layer dense transformer (CustomGPT1).

Full-input contract: kernel(**inputs) takes the unsharded inputs (B=16),
shards batch across 8 NeuronCores (2 examples/core, data-parallel; params
replicated), runs one SPMD Bass kernel, and gathers the full output.

Layout strategy per core/example:
- LN output h kept both s-major (h_nat, rhs of probs@h) and d-major (hT, both
  operands of h@h^T) in fp32r; scores computed directly in [t, s] orientation
  so the exp tiles feed probs@h with no transposes of probs.
- Softmax without max-subtraction (|scores| <= sqrt(D) by Cauchy-Schwarz after
  LN, exp fits fp32 comfortably); denominators accumulated on DVE and reduced
  across partitions with a single ones-matmul per s-chunk.
- FFN computed in f-major orientation; all matmul operands fp32r.
"""
import sys
sys.path.insert(0, "/opt/trn_rl_repo")
import math
import numpy as np
import concourse.bass as bass
import concourse.mybir as mybir
import concourse.tile as tile
from concourse import bacc
from concourse.bass_utils import run_bass_kernel_spmd
from concourse.masks import make_identity

F32 = mybir.dt.float32
F32R = mybir.dt.float32r
I32 = mybir.dt.int32
AF = mybir.ActivationFunctionType
OP = mybir.AluOpType

B, S, D, L, FF, V = 16, 2048, 512, 4, 2048, 33
NCORES, BL = 8, B // 8          # 2 examples per core
P = 128
SB = S // P                     # 16 s-blocks per example
DC = D // P                     # 4 d-chunks
FC = FF // P                    # 16 f-chunks
ASC = 512                       # attention s-chunk width
NASC = S // ASC                 # 4
ASB = ASC // P                  # 4 s-blocks per attention chunk
FSC = 512                       # ffn s-chunk width
NFSC = S // FSC                 # 4
SCALE = 1.0 / math.sqrt(D)
EPS = 1e-5
VP = V + 1                      # psum-friendly padded vocab


def build(simple):
    nc = bacc.Bacc(None, target_bir_lowering=False)

    ids = nc.dram_tensor("ids", [BL, S], I32, kind="ExternalInput")
    aidx = nc.dram_tensor("aidx", [BL], I32, kind="ExternalInput")
    mask = nc.dram_tensor("mask", [BL, S], F32, kind="ExternalInput")
    tok_emb = nc.dram_tensor("tok_emb", [V, D], F32, kind="ExternalInput")
    pos_emb = nc.dram_tensor("pos_emb", [S, D], F32, kind="ExternalInput")
    attr_emb = nc.dram_tensor("attr_emb", [608, D], F32, kind="ExternalInput")
    lnw = nc.dram_tensor("lnw", [L, D], F32, kind="ExternalInput")
    lnb = nc.dram_tensor("lnb", [L, D], F32, kind="ExternalInput")
    w1 = nc.dram_tensor("w1", [L, D, FF], F32, kind="ExternalInput")
    b1 = nc.dram_tensor("b1", [L, FF], F32, kind="ExternalInput")
    w2 = nc.dram_tensor("w2", [L, FF, D], F32, kind="ExternalInput")
    b2 = nc.dram_tensor("b2", [L, D], F32, kind="ExternalInput")
    out_w = nc.dram_tensor("out_w", [D, V], F32, kind="ExternalInput")
    out_b = nc.dram_tensor("out_b", [V], F32, kind="ExternalInput")
    out = nc.dram_tensor("out", [BL, S, V], F32, kind="ExternalOutput")

    def bcast_row(handle, offset, n):
        # [n]-vector at element `offset`, replicated across all 128 partitions
        return bass.AP(tensor=handle.ap().tensor, offset=offset, ap=[[0, P], [1, n]])

    with tile.TileContext(nc) as tc:
        with tc.tile_pool(name="cst", bufs=1) as cst, \
             tc.tile_pool(name="parw", bufs=2) as parw, \
             tc.tile_pool(name="wts", bufs=1) as wts, \
             tc.tile_pool(name="big", bufs=1) as big, \
             tc.tile_pool(name="tr4", bufs=1) as tr4, \
             tc.tile_pool(name="tmp", bufs=2) as tmp, \
             tc.tile_pool(name="sml", bufs=6) as sml, \
             tc.tile_pool(name="dram", bufs=1, space="DRAM") as dram, \
             tc.tile_pool(name="pb", bufs=6, space="PSUM") as pb, \
             tc.tile_pool(name="tpr", bufs=2, space="PSUM") as tpr:

            xbuf = dram.tile([BL, S, D], F32, tag="xbuf")
            abuf = dram.tile([BL, S, D], F32, tag="abuf")

            # constants
            ident_f = cst.tile([P, P], F32, tag="identf")
            make_identity(nc, ident_f)
            ident_r = cst.tile([P, P], F32R, tag="identr")
            nc.vector.tensor_copy(ident_r, ident_f)
            eps_t = cst.tile([P, 1], F32, tag="eps")
            nc.vector.memset(eps_t, EPS)
            ones_f = cst.tile([P, 2], F32, tag="onesf")
            nc.vector.memset(ones_f, 1.0)
            ones_r = cst.tile([P, 2], F32R, tag="onesr")
            nc.vector.tensor_copy(ones_r, ones_f)
            outb_b = cst.tile([P, V], F32, tag="outb")
            nc.sync.dma_start(out=outb_b, in_=bcast_row(out_b, 0, V))
            outw_st = cst.tile([P, DC, VP], F32, tag="outwst")
            nc.vector.memset(outw_st, 0.0)
            nc.sync.dma_start(out=outw_st[:, :, :V], in_=out_w.ap().rearrange("(do p) v -> p do v", p=P))
            outw_sb = cst.tile([P, DC, VP], F32R, tag="outw")
            nc.vector.tensor_copy(outw_sb, outw_st)

            # per-example mask bias: (m - 1) * 1e9  ==  (1 - m) * (-1e9), layout [t_in=128, tc=16]
            maskb = []
            for b in range(BL):
                ml = sml.tile([P, SB], F32, tag=f"mload{b}")
                nc.sync.dma_start(out=ml, in_=mask.ap()[b].rearrange("(tc p) -> p tc", p=P))
                mb = cst.tile([P, SB], F32, tag=f"maskb{b}")
                nc.vector.tensor_scalar(out=mb, in0=ml, scalar1=1.0, scalar2=1e9,
                                        op0=OP.subtract, op1=OP.mult)
                maskb.append(mb)

            # per-example attribute-embedding rows (replicated over partitions)
            attrvs = []
            for b in range(BL):
                ai = sml.tile([P, 1], I32, tag="aidx", name=f"ai{b}")
                nc.sync.dma_start(out=ai, in_=bass.AP(tensor=aidx.ap().tensor, offset=b, ap=[[0, P], [1, 1]]))
                attrv = cst.tile([P, D], F32, tag=f"attrv{b}")
                nc.gpsimd.indirect_dma_start(
                    out=attrv[:, :], out_offset=None, in_=attr_emb[:, :],
                    in_offset=bass.IndirectOffsetOnAxis(ap=ai[:, :1], axis=0))
                attrvs.append(attrv)

            def embed_tile(b, sb):
                """Compute x = tok_emb[ids] + pos_emb + attr row for one s-block."""
                r0 = sb * P
                it = sml.tile([P, 1], I32, tag="ids", name="it")
                nc.sync.dma_start(out=it, in_=ids.ap()[b, r0:r0 + P].rearrange("(p one) -> p one", one=1))
                tokv = tmp.tile([P, D], F32, tag="tokv", name="tokv")
                nc.gpsimd.indirect_dma_start(
                    out=tokv[:, :], out_offset=None, in_=tok_emb[:, :],
                    in_offset=bass.IndirectOffsetOnAxis(ap=it[:, :1], axis=0))
                xe = tmp.tile([P, D], F32, tag="xld", bufs=3, name="xe")
                nc.sync.dma_start(out=xe, in_=pos_emb.ap()[r0:r0 + P, :])
                nc.vector.tensor_tensor(out=xe, in0=xe, in1=tokv, op=OP.add)
                nc.vector.tensor_tensor(out=xe, in0=xe, in1=attrvs[b], op=OP.add)
                nc.sync.dma_start(out=xbuf[b, r0:r0 + P, :], in_=xe)
                return xe

            def stats_into(mvall, xt, sb):
                """bn stats of one [128, D] tile into mvall[:, sb, :] (runs inline
                with the producing phase, off the critical path)."""
                st = sml.tile([P, 6], F32, tag="st", name="st")
                nc.vector.bn_stats(st, xt)
                nc.vector.bn_aggr(mvall[:, sb, :], st)

            def finish_stats(mvall):
                """One batched sqrt + reciprocal once all 16 blocks' stats exist."""
                rstdall = sml.tile([P, SB], F32, tag="rstdall", bufs=4, name="rstdall")
                nc.scalar.activation(rstdall, mvall[:, :, 1:2], AF.Sqrt, bias=eps_t, scale=1.0)
                nc.vector.reciprocal(rstdall, rstdall)
                return rstdall

            def ln_apply(xt, mvall, rstdall, sb, lnw_b, lnb_b, out_slice):
                nc.vector.tensor_scalar(out=out_slice, in0=xt, scalar1=mvall[:, sb, 0:1],
                                        scalar2=rstdall[:, sb:sb + 1],
                                        op0=OP.subtract, op1=OP.mult)
                if not simple:
                    nc.vector.tensor_tensor(out=out_slice, in0=out_slice, in1=lnw_b, op=OP.mult)
                    nc.vector.tensor_tensor(out=out_slice, in0=out_slice, in1=lnb_b, op=OP.add)

            def transpose_to(dst, src_tile, sb):
                """PE-transpose [s128, D] fp32r tile into dst[:, :, sb*128:(sb+1)*128]."""
                r0 = sb * P
                pt = tpr.tile([P, D], F32R, tag="tpr", name="pt")
                for dc in range(DC):
                    nc.tensor.transpose(pt[:, dc * P:(dc + 1) * P], src_tile[:, dc * P:(dc + 1) * P], ident_r)
                nc.vector.tensor_copy(dst[:, :, r0:r0 + P], pt.rearrange("p (dc q) -> p dc q", q=P))

            # ---------------- layers ----------------
            mv1_next = {}   # per-example LN1 stats tile, written by the previous layer
            for l in range(L):
                w1sb = wts.tile([P, DC, FF], F32R, tag="w1")
                nc.gpsimd.dma_start(out=w1sb, in_=w1.ap()[l].rearrange("(do p) f -> p do f", p=P))
                w2sb = wts.tile([P, FC, D], F32R, tag="w2")
                nc.gpsimd.dma_start(out=w2sb, in_=w2.ap()[l].rearrange("(fc p) d -> p fc d", p=P))
                if not simple:
                    lnw_b = parw.tile([P, D], F32, tag="lnw")
                    nc.sync.dma_start(out=lnw_b, in_=bcast_row(lnw, l * D, D))
                    lnb_b = parw.tile([P, D], F32, tag="lnb")
                    nc.sync.dma_start(out=lnb_b, in_=bcast_row(lnb, l * D, D))
                    b2sb = parw.tile([P, DC], F32, tag="b2")
                    nc.sync.dma_start(out=b2sb, in_=b2.ap()[l].rearrange("(dc p) -> p dc", p=P))
                else:
                    lnw_b = lnb_b = b2sb = None
                b1sb = parw.tile([P, FC], F32, tag="b1")
                nc.sync.dma_start(out=b1sb, in_=b1.ap()[l].rearrange("(fc p) -> p fc", p=P))

                for b in range(BL):
                    # ---- LN1 (+ embeddings on layer 0): -> h_nat (s-major) + hT (d-major)
                    # Stats for LN1 were computed inline by the previous layer's FFN
                    # epilogue (or by the embedding pass on layer 0) -> only
                    # sqrt+recip+apply remain on the phase boundary.
                    h_nat = big.tile([P, SB, D], F32R, tag="h")
                    hT = tr4.tile([P, DC, S], F32R, tag="tr4", name="hT")
                    if l == 0:
                        mvall = sml.tile([P, SB, 2], F32, tag=f"mv1_{b}", bufs=2, name="mv1e")
                        for sb in range(SB):
                            xe = embed_tile(b, sb)
                            stats_into(mvall, xe, sb)
                    else:
                        mvall = mv1_next[b]
                    rstdall = finish_stats(mvall)
                    for sb in range(SB):
                        r0 = sb * P
                        xt = tmp.tile([P, D], F32, tag="xld", bufs=3, name="xt")
                        nc.sync.dma_start(out=xt, in_=xbuf[b, r0:r0 + P, :])
                        hs = h_nat[:, sb, :]
                        ln_apply(xt, mvall, rstdall, sb, lnw_b, lnb_b, hs)
                        transpose_to(hT, hs, sb)

                    # ---- attention: scoresT -> exp -> attnU accumulation; DVE denominators
                    # LN2 stats are computed inline on the attention-output tiles.
                    mvall2 = sml.tile([P, SB, 2], F32, tag="mv2", bufs=2, name="mv2")
                    for sc in range(NASC):
                        c0 = sc * ASC
                        pa = [pb.tile([P, FSC], F32, tag="pb", name=f"pa{_h}") for _h in range(ASB)]
                        dacc = tmp.tile([P, ASC], F32R, tag="dacc", name="dacc")
                        for tc_i in range(SB):
                            ps_sc = pb.tile([P, ASC], F32, tag="pb", name="ps_sc")
                            for do in range(DC):
                                nc.tensor.matmul(ps_sc, hT[:, do, tc_i * P:(tc_i + 1) * P],
                                                 hT[:, do, c0:c0 + ASC],
                                                 start=(do == 0), stop=(do == DC - 1))
                            et = tmp.tile([P, ASC], F32R, tag="expt", bufs=3, name="et")
                            nc.scalar.activation(et, ps_sc, AF.Exp,
                                                 bias=maskb[b][:, tc_i:tc_i + 1], scale=SCALE)
                            if tc_i == 0:
                                nc.vector.tensor_copy(dacc, et)
                            else:
                                nc.vector.tensor_tensor(out=dacc, in0=dacc, in1=et, op=OP.add)
                            for hf in range(ASB):
                                nc.tensor.matmul(pa[hf][:, :], et[:, hf * P:(hf + 1) * P],
                                                 h_nat[:, tc_i, :],
                                                 start=(tc_i == 0), stop=(tc_i == SB - 1))
                        # denominators: reduce dacc over partitions with a ones-matmul,
                        # then reshape the [1, ASC] row into per-partition scalars [128, ASB]
                        pden = pb.tile([1, ASC], F32, tag="pb", name="pden")
                        nc.tensor.matmul(pden, ones_r[:, :1], dacc, start=True, stop=True)
                        drow = sml.tile([1, ASC], F32, tag="drow", bufs=2, name="drow")
                        nc.vector.tensor_copy(drow, pden)
                        dsb = sml.tile([P, ASB], F32, tag="dsb", name="dsb")
                        for _hf in range(ASB):
                            nc.sync.dma_start(
                                out=dsb[:, _hf:_hf + 1],
                                in_=drow[0:1, _hf * P:(_hf + 1) * P].rearrange("one (p o) -> one p o", o=1))
                        drec = sml.tile([P, ASB], F32, tag="drec", name="drec")
                        nc.vector.reciprocal(drec, dsb)
                        for hf in range(ASB):
                            r0 = c0 + hf * P
                            at = tmp.tile([P, D], F32, tag="attn", name="at")
                            nc.vector.tensor_scalar(out=at, in0=pa[hf][:, :], scalar1=drec[:, hf:hf + 1],
                                                    scalar2=None, op0=OP.mult)
                            xr = tmp.tile([P, D], F32, tag="xres", name="xr")
                            nc.sync.dma_start(out=xr, in_=xbuf[b, r0:r0 + P, :])
                            nc.vector.tensor_tensor(out=at, in0=at, in1=xr, op=OP.add)
                            stats_into(mvall2, at, sc * ASB + hf)
                            nc.sync.dma_start(out=abuf[b, r0:r0 + P, :], in_=at)

                    # ---- LN2: abuf -> n2T (d-major, fp32r)
                    n2T = tr4.tile([P, DC, S], F32R, tag="tr4", name="n2T")
                    rstdall2 = finish_stats(mvall2)
                    for sb in range(SB):
                        r0 = sb * P
                        xt = tmp.tile([P, D], F32, tag="xld", bufs=3, name="xt2")
                        nc.sync.dma_start(out=xt, in_=abuf[b, r0:r0 + P, :])
                        n2 = tmp.tile([P, D], F32R, tag="n2", name="n2")
                        ln_apply(xt, mvall2, rstdall2, sb, lnw_b, lnb_b, n2[:, :])
                        transpose_to(n2T, n2, sb)

                    # ---- FFN (f-major): ff = gelu(n2 @ w1 + b1); x = ff @ w2 + b2 + attn
                    last = (l == L - 1)
                    if not last:
                        mv1_next[b] = sml.tile([P, SB, 2], F32, tag=f"mv1_{b}", bufs=2,
                                               name=f"mv1n{b}")
                    for fs in range(NFSC):
                        c0 = fs * FSC
                        p2 = [pb.tile([P, FSC], F32, tag="pb", name=f"p2_{_d}") for _d in range(DC)]
                        for fc in range(FC):
                            pf = pb.tile([P, FSC], F32, tag="pb", name="pf")
                            for do in range(DC):
                                nc.tensor.matmul(pf, w1sb[:, do, fc * P:(fc + 1) * P],
                                                 n2T[:, do, c0:c0 + FSC],
                                                 start=(do == 0), stop=(do == DC - 1))
                            fg = tmp.tile([P, FSC], F32R, tag="ffg", bufs=3, name="fg")
                            nc.scalar.activation(fg, pf, AF.Gelu, bias=b1sb[:, fc:fc + 1], scale=1.0)
                            for dc in range(DC):
                                nc.tensor.matmul(p2[dc], w2sb[:, fc, dc * P:(dc + 1) * P], fg,
                                                 start=(fc == 0), stop=(fc == FC - 1))
                        f2sb = []
                        for dc in range(DC):
                            t = tmp.tile([P, FSC], F32R, tag="f2sb", bufs=4, name=f"f2sb{dc}")
                            if simple:
                                nc.vector.tensor_copy(t, p2[dc])
                            else:
                                nc.vector.tensor_scalar(out=t, in0=p2[dc], scalar1=b2sb[:, dc:dc + 1],
                                                        scalar2=None, op0=OP.add)
                            f2sb.append(t)
                        for sbi in range(FSC // P):
                            r0 = c0 + sbi * P
                            px = pb.tile([P, D], F32R, tag="pb", name="px")
                            for dc in range(DC):
                                nc.tensor.transpose(px[:, dc * P:(dc + 1) * P],
                                                    f2sb[dc][:, sbi * P:(sbi + 1) * P], ident_r)
                            ar = tmp.tile([P, D], F32, tag="ares", name="ar")
                            nc.sync.dma_start(out=ar, in_=abuf[b, r0:r0 + P, :])
                            if not last:
                                xn = tmp.tile([P, D], F32, tag="xn", name="xn")
                                nc.vector.tensor_tensor(out=xn, in0=px, in1=ar, op=OP.add)
                                stats_into(mv1_next[b], xn, r0 // P)  # LN1 stats for layer l+1
                                nc.sync.dma_start(out=xbuf[b, r0:r0 + P, :], in_=xn)
                            else:
                                # final projection fused into the epilogue
                                xnr = tmp.tile([P, D], F32R, tag="xn", name="xnr")
                                nc.vector.tensor_tensor(out=xnr, in0=px, in1=ar, op=OP.add)
                                pt = tpr.tile([P, D], F32R, tag="tpr", name="ptx")
                                for dc in range(DC):
                                    nc.tensor.transpose(pt[:, dc * P:(dc + 1) * P],
                                                        xnr[:, dc * P:(dc + 1) * P], ident_r)
                                xtsb = tmp.tile([P, DC, P], F32R, tag="xtsb", name="xtsb")
                                nc.vector.tensor_copy(xtsb, pt.rearrange("p (dc q) -> p dc q", q=P))
                                po = pb.tile([P, VP], F32, tag="pb", name="po")
                                for do in range(DC):
                                    nc.tensor.matmul(po, xtsb[:, do, :], outw_sb[:, do, :],
                                                     start=(do == 0), stop=(do == DC - 1))
                                ot = tmp.tile([P, V], F32, tag="ot", name="ot")
                                nc.vector.tensor_tensor(out=ot, in0=po[:, :V], in1=outb_b, op=OP.add)
                                nc.sync.dma_start(out=out[b, r0:r0 + P, :], in_=ot)

    nc.compile()
    return nc


_NC = {}


def _get_nc(simple=True):
    if simple not in _NC:
        _NC[simple] = build(simple)
    return _NC[simple]


def _is_simple(inputs):
    return (np.all(np.asarray(inputs["ln_w"]) == 1.0)
            and np.all(np.asarray(inputs["ln_b"]) == 0.0)
            and np.all(np.asarray(inputs["b2"]) == 0.0))


def make_in_maps(inputs):
    f = lambda a: np.ascontiguousarray(np.asarray(a, dtype=np.float32))
    i = lambda a: np.ascontiguousarray(np.asarray(a, dtype=np.int32))
    shared = {
        "tok_emb": f(inputs["tok_emb"]), "pos_emb": f(inputs["pos_emb"]),
        "attr_emb": f(inputs["attr_emb"]),
        "lnw": f(inputs["ln_w"]), "lnb": f(inputs["ln_b"]),
        "w1": f(inputs["w1"]), "b1": f(inputs["b1"]),
        "w2": f(inputs["w2"]), "b2": f(inputs["b2"]),
        "out_w": f(inputs["out_w"]), "out_b": f(inputs["out_b"]),
    }
    in_maps = []
    for c in range(NCORES):
        sl = slice(BL * c, BL * (c + 1))
        m = dict(shared)
        m["ids"] = i(inputs["input_ids"][sl])
        m["aidx"] = i(inputs["combined_indices"][sl])
        m["mask"] = f(inputs["attention_mask"][sl])
        in_maps.append(m)
    return in_maps


def kernel(**inputs):
    res = run_bass_kernel_spmd(_get_nc(_is_simple(inputs)), make_in_maps(inputs),
                               core_ids=list(range(NCORES)))
    return np.concatenate([r["out"] for r in res.results], axis=0)
